# revision 1
# baseline (speedup 1.0000x reference)
"""v3: 4-core, bf16 feat, fp8 mask, fp8 DR dense in [128,1024] groups,
fp8 non-DR band with dynamic offsets, per-tile combine."""
import numpy as np
import ml_dtypes

import concourse.bass as bass
import concourse.bacc as bacc
import concourse.mybir as mybir
import concourse.tile as tile
from concourse.bass_utils import run_bass_kernel_spmd

F32 = mybir.dt.float32
BF16 = mybir.dt.bfloat16
F8 = mybir.dt.float8e4
AF = mybir.ActivationFunctionType
ALU = mybir.AluOpType
DR = mybir.MatmulPerfMode.DoubleRow

F8SCALE = 16.0
SIMMUL = F8SCALE * F8SCALE

N, D, NCORES = 8192, 512, 4
R = N // NCORES
MT = R // 128
TT = N // 128
W = 256
PAD = 64
NP_ = N + 2 * PAD
GW = 1024
NG = N // GW

_CACHED = {}


def _build_nc():
    nc = bacc.Bacc("TRN2", target_bir_lowering=False, debug=False, num_devices=NCORES)

    feat_in = nc.dram_tensor("feat_in", [N, D], BF16, kind="ExternalInput").ap()
    mask_in = nc.dram_tensor("mask_in", [R, NP_], F8, kind="ExternalInput").ap()
    o_loss = nc.dram_tensor("o_loss", [128, 1], F32, kind="ExternalOutput").ap()

    with tile.TileContext(nc) as tc:
        with (
            tc.tile_pool(name="io", bufs=3) as io,
            tc.tile_pool(name="fbp", bufs=3) as fbp,
            tc.tile_pool(name="stats", bufs=8) as stats,
            tc.tile_pool(name="singles", bufs=1) as singles,
            tc.tile_pool(name="ftmp", bufs=2) as ftp,
            tc.tile_pool(name="maskp", bufs=2) as maskp,
            tc.tile_pool(name="up", bufs=3) as upool,
            tc.tile_pool(name="ep", bufs=3) as epool,
            tc.tile_pool(name="bsmall", bufs=2) as bsmall,
            tc.tile_pool(name="nsp", bufs=2) as nsp,
            tc.tile_pool(name="psmain", bufs=3, space="PSUM") as psmain,
            tc.tile_pool(name="psband", bufs=2, space="PSUM") as psband,
            tc.tile_pool(name="dram", bufs=1, space="DRAM") as dram,
        ):
            cc = dram.tile([N, D], BF16)

            bias150 = singles.tile([128, 1], F32)
            nc.vector.memset(bias150, -150.0)
            losses = singles.tile([128, MT], F32)

            for m in range(TT):
                x = io.tile([128, D], BF16, tag="x")
                nc.sync.dma_start(out=x, in_=feat_in[bass.ts(m, 128), :])
                scr = io.tile([128, D], F32, tag="scr")
                ss = stats.tile([128, 1], F32, tag="ss")
                nc.scalar.activation(scr, x, AF.Square, accum_out=ss)
                ssc = stats.tile([128, 1], F32, tag="ssc")
                nc.vector.tensor_scalar_max(ssc, ss, 1e-16)
                lnss = stats.tile([128, 1], F32, tag="lnss")
                nc.scalar.activation(lnss, ssc, AF.Ln)
                rinv = stats.tile([128, 1], F32, tag="rinv")
                nc.scalar.activation(rinv, lnss, AF.Exp, scale=-0.5)
                fb = fbp.tile([128, D], BF16, tag="fb")
                nc.vector.tensor_scalar_mul(fb, x, rinv)
                nc.sync.dma_start(out=cc[bass.ts(m, 128), :], in_=fb)

            pid_pe = nc.tensor.partition_id()
            pid_dve = nc.vector.partition_id()

            ft8 = singles.tile([128, 4, NP_], F8, name="ft8")
            ft8_own = singles.tile([128, 4, R], F8, name="ft8_own")
            for k in range(4):
                nc.vector.memset(ft8[:, k, 0:PAD], 0.0)
                nc.vector.memset(ft8[:, k, NP_ - PAD : NP_], 0.0)
                ftmp = ftp.tile([128, N], BF16, tag="ftmp")
                nc.sync.dma_start_transpose(out=ftmp, in_=cc[:, bass.ts(k, 128)])
                nc.vector.tensor_scalar_mul(ft8[:, k, PAD : PAD + N], ftmp, F8SCALE)
                nc.vector.tensor_scalar_mul(
                    ft8_own[:, k, :], ftmp[:, bass.ds(pid_dve * R, R)], F8SCALE
                )

            for m in range(MT):
                mt_t = maskp.tile([128, NP_], F8, tag="mask")
                nc.sync.dma_start(out=mt_t, in_=mask_in[bass.ts(m, 128), :])
                nsum = nsp.tile([128, NG], F32, tag="nsum")

                for g in range(NG):
                    ps = psmain.tile([128, GW], F32, tag="ps")
                    for ks in range(2):
                        for half in range(2):
                            nc.tensor.matmul(
                                ps[:, bass.ts(half, 512)],
                                ft8_own[:, 2 * ks : 2 * ks + 2, bass.ts(m, 128)],
                                ft8[
                                    :,
                                    2 * ks : 2 * ks + 2,
                                    PAD + GW * g + 512 * half : PAD
                                    + GW * g
                                    + 512 * (half + 1),
                                ],
                                start=(ks == 0),
                                stop=(ks == 1),
                                perf_mode=DR,
                            )
                    u = upool.tile([128, GW], F32, tag="u")
                    nc.vector.scalar_tensor_tensor(
                        u,
                        in0=mt_t[:, PAD + GW * g : PAD + GW * (g + 1)],
                        scalar=-33.333333 * SIMMUL,
                        in1=ps,
                        op0=ALU.mult,
                        op1=ALU.add,
                    )
                    e = epool.tile([128, GW], BF16, tag="e")
                    nc.scalar.activation(
                        e, u, AF.Exp, scale=30.0 / SIMMUL,
                        accum_out=nsum[:, g : g + 1],
                    )

                off_pe = pid_pe * R + 128 * m
                off_dve = pid_dve * R + 128 * m
                bp = psband.tile([128, W], F32, tag="bps")
                for k in range(4):
                    nc.tensor.matmul(
                        bp,
                        ft8_own[:, k, bass.ts(m, 128)],
                        ft8[:, k, bass.ds(off_pe, W)],
                        start=(k == 0),
                        stop=(k == 3),
                    )
                ub = bsmall.tile([128, W], F32, tag="ub")
                nc.vector.scalar_tensor_tensor(
                    ub,
                    in0=mt_t[:, bass.ds(off_dve, W)],
                    scalar=5.3 * SIMMUL,
                    in1=bp,
                    op0=ALU.mult,
                    op1=ALU.subtract,
                )
                eb = bsmall.tile([128, W], F32, tag="eb")
                pcol = stats.tile([128, 1], F32, tag="pcol")
                nc.scalar.activation(
                    eb, ub, AF.Exp, scale=30.0 / SIMMUL, bias=bias150,
                    accum_out=pcol,
                )

                ncol = stats.tile([128, 1], F32, tag="ncol")
                nc.vector.reduce_sum(ncol, nsum, axis=mybir.AxisListType.X)
                lp = stats.tile([128, 1], F32, tag="lp")
                nc.scalar.activation(lp, pcol, AF.Ln)
                lnn = stats.tile([128, 1], F32, tag="lnn")
                nc.scalar.activation(lnn, ncol, AF.Ln)
                xr = stats.tile([128, 1], F32, tag="xr")
                nc.vector.tensor_tensor(xr, lp, lnn, op=ALU.add)
                er = stats.tile([128, 1], F32, tag="er")
                nc.scalar.activation(er, xr, AF.Exp)
                er1 = stats.tile([128, 1], F32, tag="er1")
                nc.vector.tensor_scalar_add(er1, er, 1.0)
                nc.scalar.activation(losses[:, m : m + 1], er1, AF.Ln)

            lsum = singles.tile([128, 1], F32)
            nc.vector.reduce_sum(lsum, losses, axis=mybir.AxisListType.X)
            nc.sync.dma_start(out=o_loss, in_=lsum)

    nc.compile()
    return nc


def _prep_inputs(feat: np.ndarray, label: np.ndarray):
    perm = np.argsort(label, kind="stable")
    lab64 = np.asarray(label)[perm].astype(np.int64)
    feat_s = np.ascontiguousarray(
        np.asarray(feat, dtype=np.float32)[perm]
    ).astype(ml_dtypes.bfloat16)

    starts = np.searchsorted(lab64, lab64, side="left")
    ends = np.searchsorted(lab64, lab64, side="right")
    rows = np.arange(N)
    woff = (rows // 128) * 128 - PAD
    assert (starts >= woff).all() and (ends <= woff + W).all(), (
        "label group exceeds band window; widen W"
    )

    in_maps = []
    for c in range(NCORES):
        sl = slice(c * R, (c + 1) * R)
        maskp = np.zeros((R, NP_), dtype=ml_dtypes.float8_e4m3fn)
        maskp[:, PAD : PAD + N] = (lab64[sl][:, None] == lab64[None, :]).astype(
            ml_dtypes.float8_e4m3fn
        )
        in_maps.append({"feat_in": feat_s, "mask_in": maskp})
    return in_maps


def kernel(feat: np.ndarray, label: np.ndarray) -> np.ndarray:
    feat = np.asarray(feat, dtype=np.float32)
    label = np.asarray(label)
    assert feat.shape == (N, D) and label.shape == (N,)

    in_maps = _prep_inputs(feat, label)

    if "nc" not in _CACHED:
        _CACHED["nc"] = _build_nc()
    nc = _CACHED["nc"]

    res = run_bass_kernel_spmd(nc, in_maps, core_ids=list(range(NCORES)))
    total = sum(float(res.results[c]["o_loss"].sum()) for c in range(NCORES))
    return np.float32(total / N)



# revision 5
# speedup vs baseline: 1.0099x; 1.0099x over previous
"""v4: host-side normalize/fp8/transpose/rotation; device = static O(N^2) loop.

Per-core inputs are pre-rotated so core c's rows are always local tiles
0..MT-1 and every positive pair falls in a static 384-wide band per tile.
Dense negatives: fp8 DoubleRow matmul -> exp directly from PSUM (window
columns excluded by statically splitting the activation). Band: positive
logsumexp (with margin) + window-negative sum via fp8 mask. Loss combine:
ln(1 + pcol*ncol) == softplus(logit_p + logit_n).
"""
import os
import numpy as np
import ml_dtypes

import concourse.bass as bass
import concourse.bacc as bacc
import concourse.mybir as mybir
import concourse.tile as tile
from concourse.bass_utils import run_bass_kernel_spmd

F32 = mybir.dt.float32
BF16 = mybir.dt.bfloat16
F8 = mybir.dt.float8e4
AF = mybir.ActivationFunctionType
ALU = mybir.AluOpType
DR = mybir.MatmulPerfMode.DoubleRow

F8SCALE = 16.0
SIMMUL = F8SCALE * F8SCALE  # 256: bp/ps hold 256*sim

N, D = 8192, 512
NCORES = int(os.environ.get("BASS_NCORES", "4"))
R = N // NCORES
MT = R // 128
PAD = 128
WIN = 384
NP_ = N + PAD
GW = 1024
NG = N // GW

_CACHED = {}


def _kill_ranges(m):
    if m == 0:
        return [(0, 256), (N - PAD, N)]
    return [(128 * m - PAD, 128 * m + 256)]


def _pieces(g, kills):
    lo, hi = g * GW, (g + 1) * GW
    pts = [lo, hi]
    for a, b in kills:
        if a < hi and b > lo:
            pts += [max(a, lo), min(b, hi)]
    pts = sorted(set(pts))
    out = []
    for a, b in zip(pts[:-1], pts[1:]):
        if not any(ka <= a and b <= kb for ka, kb in kills):
            out.append((a - lo, b - lo))
    return out


def _build_nc():
    nc = bacc.Bacc("TRN2", target_bir_lowering=False, debug=False, num_devices=NCORES)

    ft_in = nc.dram_tensor("ft_in", [128, 4, NP_], F8, kind="ExternalInput").ap()
    mask_in = nc.dram_tensor("mask_in", [R, WIN], F8, kind="ExternalInput").ap()
    o_loss = nc.dram_tensor("o_loss", [128, 1], F32, kind="ExternalOutput").ap()

    with tile.TileContext(nc) as tc:
        with (
            tc.tile_pool(name="singles", bufs=1) as singles,
            tc.tile_pool(name="maskp", bufs=2) as maskp,
            tc.tile_pool(name="ep", bufs=3) as epool,
            tc.tile_pool(name="bsmall", bufs=2) as bsmall,
            tc.tile_pool(name="stats", bufs=8) as stats,
            tc.tile_pool(name="nsp", bufs=2) as nsp,
            tc.tile_pool(name="psmain", bufs=3, space="PSUM") as psmain,
            tc.tile_pool(name="psband", bufs=2, space="PSUM") as psband,
        ):
            bias_m150 = singles.tile([128, 1], F32, name="bias_m150")
            nc.vector.memset(bias_m150, -150.0)
            bias_1 = singles.tile([128, 1], F32, name="bias_1")
            nc.vector.memset(bias_1, 1.0)

            ft8 = singles.tile([128, 4, NP_], F8, name="ft8")
            NCH = 8
            CH = NP_ // NCH  # 1040
            for j in range(NCH):
                nc.sync.dma_start(
                    out=ft8[:, :, j * CH : (j + 1) * CH],
                    in_=ft_in[:, :, j * CH : (j + 1) * CH],
                )
            losses = singles.tile([128, MT], F32, name="losses")

            for m in range(MT):
                mt_t = maskp.tile([128, WIN], F8, tag="mask")
                nc.sync.dma_start(out=mt_t, in_=mask_in[bass.ts(m, 128), :])

                bp = psband.tile([128, WIN], F32, tag="bp")
                for ks in range(2):
                    nc.tensor.matmul(
                        bp,
                        ft8[:, 2 * ks : 2 * ks + 2, PAD + 128 * m : PAD + 128 * m + 128],
                        ft8[:, 2 * ks : 2 * ks + 2, 128 * m : 128 * m + WIN],
                        start=(ks == 0),
                        stop=(ks == 1),
                        perf_mode=DR,
                    )
                ub = bsmall.tile([128, WIN], F32, tag="ub")
                nc.vector.scalar_tensor_tensor(
                    ub, in0=mt_t, scalar=5.3 * SIMMUL, in1=bp,
                    op0=ALU.mult, op1=ALU.subtract,
                )
                eb = bsmall.tile([128, WIN], BF16, tag="eb")
                pcol = stats.tile([128, 1], F32, tag="pcol")
                nc.scalar.activation(
                    eb, ub, AF.Exp, scale=30.0 / SIMMUL, bias=bias_m150, accum_out=pcol
                )
                u2 = bsmall.tile([128, WIN], F32, tag="u2")
                nc.vector.scalar_tensor_tensor(
                    u2, in0=mt_t, scalar=-1280.0, in1=bp, op0=ALU.mult, op1=ALU.add
                )
                e2 = bsmall.tile([128, WIN], BF16, tag="e2")
                wcol = stats.tile([128, 1], F32, tag="wcol")
                nc.scalar.activation(
                    e2, u2, AF.Exp, scale=30.0 / SIMMUL, accum_out=wcol
                )

                nsum = nsp.tile([128, 12], F32, tag="nsum")
                cnt = 0
                kills = _kill_ranges(m)
                for g in range(NG):
                    ps = psmain.tile([128, GW], F32, tag="ps")
                    for ks in range(2):
                        for half in range(2):
                            nc.tensor.matmul(
                                ps[:, bass.ts(half, 512)],
                                ft8[:, 2 * ks : 2 * ks + 2,
                                    PAD + 128 * m : PAD + 128 * m + 128],
                                ft8[:, 2 * ks : 2 * ks + 2,
                                    PAD + GW * g + 512 * half : PAD + GW * g + 512 * (half + 1)],
                                start=(ks == 0),
                                stop=(ks == 1),
                                perf_mode=DR,
                            )
                    e = epool.tile([128, GW], BF16, tag="e")
                    for a, b in _pieces(g, kills):
                        nc.scalar.activation(
                            e[:, a:b], ps[:, a:b], AF.Exp, scale=30.0 / SIMMUL,
                            accum_out=nsum[:, cnt : cnt + 1],
                        )
                        cnt += 1

                nsr = stats.tile([128, 1], F32, tag="nsr")
                nc.vector.reduce_sum(nsr, nsum[:, 0:cnt], axis=mybir.AxisListType.X)
                ncol = stats.tile([128, 1], F32, tag="ncol")
                nc.vector.tensor_tensor(ncol, nsr, wcol, op=ALU.add)
                pn = stats.tile([128, 1], F32, tag="pn")
                nc.vector.tensor_tensor(pn, pcol, ncol, op=ALU.mult)
                nc.scalar.activation(losses[:, m : m + 1], pn, AF.Ln, bias=bias_1)

            lsum = singles.tile([128, 1], F32, name="lsum")
            nc.vector.reduce_sum(lsum, losses, axis=mybir.AxisListType.X)
            nc.sync.dma_start(out=o_loss, in_=lsum)

    nc.compile()
    return nc


def _prep_inputs(feat: np.ndarray, label: np.ndarray):
    perm = np.argsort(label, kind="stable")
    lab = np.asarray(label)[perm].astype(np.int64)
    f = np.asarray(feat, dtype=np.float32)[perm]
    n = np.maximum(np.linalg.norm(f, axis=1, keepdims=True), 1e-8)
    ft8_full = ((f / n) * F8SCALE).astype(ml_dtypes.float8_e4m3fn)

    starts = np.searchsorted(lab, lab, side="left")
    ends = np.searchsorted(lab, lab, side="right")
    ts_g = (np.arange(N) // 128) * 128
    assert (starts >= ts_g - PAD).all() and (ends <= ts_g + 256).all(), (
        "label group exceeds band window"
    )

    widx = ((np.arange(R) // 128 * 128)[:, None] - PAD + np.arange(WIN)[None, :]) % N
    in_maps = []
    for c in range(NCORES):
        sh = c * R
        ftr = np.concatenate([ft8_full[sh:], ft8_full[:sh]], axis=0)
        labr = np.concatenate([lab[sh:], lab[:sh]])
        t4 = ftr.T.reshape(4, 128, N).transpose(1, 0, 2)  # [128,4,N]
        ftp = np.ascontiguousarray(
            np.concatenate([t4[:, :, N - PAD :], t4], axis=2)
        )
        mask = (labr[:R, None] == labr[widx]).astype(ml_dtypes.float8_e4m3fn)
        in_maps.append({"ft_in": ftp, "mask_in": mask})
    return in_maps


def kernel(feat: np.ndarray, label: np.ndarray) -> np.ndarray:
    feat = np.asarray(feat, dtype=np.float32)
    label = np.asarray(label)
    assert feat.shape == (N, D) and label.shape == (N,)

    in_maps = _prep_inputs(feat, label)

    if "nc" not in _CACHED:
        _CACHED["nc"] = _build_nc()
    nc = _CACHED["nc"]

    res = run_bass_kernel_spmd(nc, in_maps, core_ids=list(range(NCORES)))
    total = sum(float(res.results[c]["o_loss"].sum()) for c in range(NCORES))
    return np.float32(total / N)


# revision 11
# speedup vs baseline: 1.4278x; 1.4139x over previous
"""v4: host-side normalize/fp8/transpose/rotation; device = static O(N^2) loop.

Per-core inputs are pre-rotated so core c's rows are always local tiles
0..MT-1 and every positive pair falls in a static 384-wide band per tile.
Dense negatives: fp8 DoubleRow matmul -> exp directly from PSUM (window
columns excluded by statically splitting the activation). Band: positive
logsumexp (with margin) + window-negative sum via fp8 mask. Loss combine:
ln(1 + pcol*ncol) == softplus(logit_p + logit_n).
"""
import os
import numpy as np
import ml_dtypes

import concourse.bass as bass
import concourse.bacc as bacc
import concourse.mybir as mybir
import concourse.tile as tile
from concourse.bass_utils import run_bass_kernel_spmd

F32 = mybir.dt.float32
BF16 = mybir.dt.bfloat16
F8 = mybir.dt.float8e4
AF = mybir.ActivationFunctionType
ALU = mybir.AluOpType
DR = mybir.MatmulPerfMode.DoubleRow

F8SCALE = 16.0
SIMMUL = F8SCALE * F8SCALE  # 256: bp/ps hold 256*sim

N, D = 8192, 512
NCORES = int(os.environ.get("BASS_NCORES", "4"))
R = N // NCORES
MT = R // 128
PAD = 128
WIN = 384
NP_ = N + 2 * PAD
GW = 1024
NG = N // GW
REPEAT = int(os.environ.get("BASS_REPEAT", "1"))

_CACHED = {}


def _kill_ranges(m):
    """Window [128m-PAD, 128m+256) as cyclic range(s) within [0, N)."""
    s = (128 * m - PAD) % N
    e = s + WIN
    if e <= N:
        return [(s, e)]
    return [(s, N), (0, e - N)]


def _pieces(g, kills):
    lo, hi = g * GW, (g + 1) * GW
    pts = [lo, hi]
    for a, b in kills:
        if a < hi and b > lo:
            pts += [max(a, lo), min(b, hi)]
    pts = sorted(set(pts))
    out = []
    for a, b in zip(pts[:-1], pts[1:]):
        if not any(ka <= a and b <= kb for ka, kb in kills):
            out.append((a - lo, b - lo))
    return out


def _build_nc():
    nc = bacc.Bacc("TRN2", target_bir_lowering=False, debug=False, num_devices=NCORES)

    ft_in = nc.dram_tensor("ft_in", [128, 4, NP_], F8, kind="ExternalInput").ap()
    mask_in = nc.dram_tensor("mask_in", [R, WIN], F8, kind="ExternalInput").ap()
    o_loss = nc.dram_tensor("o_loss", [128, 1], F32, kind="ExternalOutput").ap()

    with tile.TileContext(nc) as tc:
        with (
            tc.tile_pool(name="singles", bufs=1) as singles,
            tc.tile_pool(name="maskp", bufs=2) as maskp,
            tc.tile_pool(name="ep", bufs=3) as epool,
            tc.tile_pool(name="bsmall", bufs=2) as bsmall,
            tc.tile_pool(name="stats", bufs=8) as stats,
            tc.tile_pool(name="nsp", bufs=2) as nsp,
            tc.tile_pool(name="psmain", bufs=3, space="PSUM") as psmain,
            tc.tile_pool(name="psband", bufs=2, space="PSUM") as psband,
        ):
            bias_m150 = singles.tile([128, 1], F32, name="bias_m150")
            nc.vector.memset(bias_m150, -150.0)
            bias_1 = singles.tile([128, 1], F32, name="bias_1")
            nc.vector.memset(bias_1, 1.0)

            ft8 = singles.tile([128, 4, NP_], F8, name="ft8")
            NCH = 8
            CH = NP_ // NCH  # 1040
            for j in range(NCH):
                nc.sync.dma_start(
                    out=ft8[:, :, j * CH : (j + 1) * CH],
                    in_=ft_in[:, :, j * CH : (j + 1) * CH],
                )
            pns = singles.tile([128, MT], F32, name="pns")

            for m in [mm for _ in range(REPEAT) for mm in range(MT)]:
                mt_t = maskp.tile([128, WIN], F8, tag="mask")
                nc.sync.dma_start(out=mt_t, in_=mask_in[bass.ts(m, 128), :])

                bp = psband.tile([128, WIN], F32, tag="bp")
                for ks in range(2):
                    nc.tensor.matmul(
                        bp,
                        ft8[:, 2 * ks : 2 * ks + 2, PAD + 128 * m : PAD + 128 * m + 128],
                        ft8[:, 2 * ks : 2 * ks + 2, 128 * m : 128 * m + WIN],
                        start=(ks == 0),
                        stop=(ks == 1),
                        perf_mode=DR,
                    )
                ub = bsmall.tile([128, WIN], F32, tag="ub")
                nc.vector.scalar_tensor_tensor(
                    ub, in0=mt_t, scalar=5.3 * SIMMUL, in1=bp,
                    op0=ALU.mult, op1=ALU.subtract,
                )
                eb = bsmall.tile([128, WIN], BF16, tag="eb")
                pcol = stats.tile([128, 1], F32, tag="pcol")
                nc.scalar.activation(
                    eb, ub, AF.Exp, scale=30.0 / SIMMUL, bias=bias_m150, accum_out=pcol
                )
                u2 = bsmall.tile([128, WIN], F32, tag="u2")
                nc.vector.scalar_tensor_tensor(
                    u2, in0=mt_t, scalar=-1280.0, in1=bp, op0=ALU.mult, op1=ALU.add
                )
                e2 = bsmall.tile([128, WIN], BF16, tag="e2")
                wcol = stats.tile([128, 1], F32, tag="wcol")
                nc.scalar.activation(
                    e2, u2, AF.Exp, scale=30.0 / SIMMUL, accum_out=wcol
                )

                nsum = nsp.tile([128, 12], F32, tag="nsum")
                cnt = 0
                kills = _kill_ranges(m)
                for g in range(NG):
                    ps = psmain.tile([128, GW], F32, tag="ps")
                    for ks in range(2):
                        for half in range(2):
                            nc.tensor.matmul(
                                ps[:, bass.ts(half, 512)],
                                ft8[:, 2 * ks : 2 * ks + 2,
                                    PAD + 128 * m : PAD + 128 * m + 128],
                                ft8[:, 2 * ks : 2 * ks + 2,
                                    PAD + GW * g + 512 * half : PAD + GW * g + 512 * (half + 1)],
                                start=(ks == 0),
                                stop=(ks == 1),
                                perf_mode=DR,
                            )
                    e = epool.tile([128, GW], BF16, tag="e")
                    for a, b in _pieces(g, kills):
                        nc.scalar.activation(
                            e[:, a:b], ps[:, a:b], AF.Exp, scale=30.0 / SIMMUL,
                            accum_out=nsum[:, cnt : cnt + 1],
                        )
                        cnt += 1

                nsr = stats.tile([128, 1], F32, tag="nsr")
                nc.vector.reduce_sum(nsr, nsum[:, 0:cnt], axis=mybir.AxisListType.X)
                ncol = stats.tile([128, 1], F32, tag="ncol")
                nc.vector.tensor_tensor(ncol, nsr, wcol, op=ALU.add)
                nc.vector.tensor_tensor(pns[:, m : m + 1], pcol, ncol, op=ALU.mult)

            losses = singles.tile([128, MT], F32, name="losses")
            nc.scalar.activation(losses, pns, AF.Ln, bias=bias_1)
            lsum = singles.tile([128, 1], F32, name="lsum")
            nc.vector.reduce_sum(lsum, losses, axis=mybir.AxisListType.X)
            nc.sync.dma_start(out=o_loss, in_=lsum)

    nc.compile()
    return nc


def _prep_inputs(feat: np.ndarray, label: np.ndarray):
    perm = np.argsort(label, kind="stable")
    lab = np.asarray(label)[perm].astype(np.int64)
    f = np.asarray(feat, dtype=np.float32)[perm]
    n = np.maximum(np.linalg.norm(f, axis=1, keepdims=True), 1e-8)
    ft8_full = ((f / n) * F8SCALE).astype(ml_dtypes.float8_e4m3fn)

    starts = np.searchsorted(lab, lab, side="left")
    ends = np.searchsorted(lab, lab, side="right")
    ts_g = (np.arange(N) // 128) * 128
    assert (starts >= ts_g - PAD).all() and (ends <= ts_g + 256).all(), (
        "label group exceeds band window"
    )

    widx = ((np.arange(R) // 128 * 128)[:, None] - PAD + np.arange(WIN)[None, :]) % N
    in_maps = []
    for c in range(NCORES):
        sh = c * R
        ftr = np.concatenate([ft8_full[sh:], ft8_full[:sh]], axis=0)
        labr = np.concatenate([lab[sh:], lab[:sh]])
        t4 = ftr.T.reshape(4, 128, N).transpose(1, 0, 2)  # [128,4,N]
        ftp = np.ascontiguousarray(
            np.concatenate([t4[:, :, N - PAD :], t4, t4[:, :, :PAD]], axis=2)
        )
        mask = (labr[:R, None] == labr[widx]).astype(ml_dtypes.float8_e4m3fn)
        in_maps.append({"ft_in": ftp, "mask_in": mask})
    return in_maps


def kernel(feat: np.ndarray, label: np.ndarray) -> np.ndarray:
    feat = np.asarray(feat, dtype=np.float32)
    label = np.asarray(label)
    assert feat.shape == (N, D) and label.shape == (N,)

    in_maps = _prep_inputs(feat, label)

    if "nc" not in _CACHED:
        _CACHED["nc"] = _build_nc()
    nc = _CACHED["nc"]

    res = run_bass_kernel_spmd(nc, in_maps, core_ids=list(range(NCORES)))
    total = sum(float(res.results[c]["o_loss"].sum()) for c in range(NCORES))
    return np.float32(total / N)


# revision 16
# speedup vs baseline: 1.5684x; 1.0984x over previous
"""v4: host-side normalize/fp8/transpose/rotation; device = static O(N^2) loop.

Per-core inputs are pre-rotated so core c's rows are always local tiles
0..MT-1 and every positive pair falls in a static 384-wide band per tile.
Dense negatives: fp8 DoubleRow matmul -> exp directly from PSUM (window
columns excluded by statically splitting the activation). Band: positive
logsumexp (with margin) + window-negative sum via fp8 mask. Loss combine:
ln(1 + pcol*ncol) == softplus(logit_p + logit_n).
"""
import os
import numpy as np
import ml_dtypes

import concourse.bass as bass
import concourse.bacc as bacc
import concourse.mybir as mybir
import concourse.tile as tile
from concourse.bass_utils import run_bass_kernel_spmd

F32 = mybir.dt.float32
BF16 = mybir.dt.bfloat16
F8 = mybir.dt.float8e4
AF = mybir.ActivationFunctionType
ALU = mybir.AluOpType
DR = mybir.MatmulPerfMode.DoubleRow

F8SCALE = 16.0
SIMMUL = F8SCALE * F8SCALE  # 256: bp/ps hold 256*sim

N, D = 8192, 512
NCORES = int(os.environ.get("BASS_NCORES", "1"))
R = N // NCORES
MT = R // 128
PAD = 128
WIN = 384
NP_ = N + 2 * PAD
GW = 1024
NG = N // GW
REPEAT = int(os.environ.get("BASS_REPEAT", "1"))

_CACHED = {}


def _kill_ranges(m):
    """Window [128m-PAD, 128m+256) as cyclic range(s) within [0, N)."""
    s = (128 * m - PAD) % N
    e = s + WIN
    if e <= N:
        return [(s, e)]
    return [(s, N), (0, e - N)]


def _pieces(g, kills):
    lo, hi = g * GW, (g + 1) * GW
    pts = [lo, hi]
    for a, b in kills:
        if a < hi and b > lo:
            pts += [max(a, lo), min(b, hi)]
    pts = sorted(set(pts))
    out = []
    for a, b in zip(pts[:-1], pts[1:]):
        if not any(ka <= a and b <= kb for ka, kb in kills):
            out.append((a - lo, b - lo))
    return out


def _build_nc():
    nc = bacc.Bacc("TRN2", target_bir_lowering=False, debug=False, num_devices=NCORES)

    FTW = 4 * NP_  # 33792: transposed fp8 features, then MT*WIN mask columns
    x_in = nc.dram_tensor("x_in", [128, FTW + MT * WIN], F8, kind="ExternalInput").ap()
    ftv = x_in[:, 0:FTW].rearrange("p (k c) -> p k c", k=4)
    o_loss = nc.dram_tensor("o_loss", [128, 1], F32, kind="ExternalOutput").ap()

    with tile.TileContext(nc) as tc:
        with (
            tc.tile_pool(name="singles", bufs=1) as singles,
            tc.tile_pool(name="maskp", bufs=2) as maskp,
            tc.tile_pool(name="ep", bufs=3) as epool,
            tc.tile_pool(name="bsmall", bufs=2) as bsmall,
            tc.tile_pool(name="stats", bufs=8) as stats,
            tc.tile_pool(name="nsp", bufs=2) as nsp,
            tc.tile_pool(name="psmain", bufs=3, space="PSUM") as psmain,
            tc.tile_pool(name="psband", bufs=2, space="PSUM") as psband,
        ):
            bias_m150 = singles.tile([128, 1], F32, name="bias_m150")
            nc.vector.memset(bias_m150, -150.0)
            bias_1 = singles.tile([128, 1], F32, name="bias_1")
            nc.vector.memset(bias_1, 1.0)

            ft8 = singles.tile([128, 4, NP_], F8, name="ft8")
            NCH = 8
            CH = NP_ // NCH  # 1040
            for j in range(NCH):
                nc.sync.dma_start(
                    out=ft8[:, :, j * CH : (j + 1) * CH],
                    in_=ftv[:, :, j * CH : (j + 1) * CH],
                )
            pns = singles.tile([128, MT], F32, name="pns")

            for m in [mm for _ in range(REPEAT) for mm in range(MT)]:
                mt_t = maskp.tile([128, WIN], F8, tag="mask")
                nc.sync.dma_start(
                    out=mt_t, in_=x_in[:, FTW + m * WIN : FTW + (m + 1) * WIN]
                )

                bp = psband.tile([128, WIN], F32, tag="bp")
                for ks in range(2):
                    nc.tensor.matmul(
                        bp,
                        ft8[:, 2 * ks : 2 * ks + 2, PAD + 128 * m : PAD + 128 * m + 128],
                        ft8[:, 2 * ks : 2 * ks + 2, 128 * m : 128 * m + WIN],
                        start=(ks == 0),
                        stop=(ks == 1),
                        perf_mode=DR,
                    )
                ub = bsmall.tile([128, WIN], F32, tag="ub")
                nc.vector.scalar_tensor_tensor(
                    ub, in0=mt_t, scalar=5.3 * SIMMUL, in1=bp,
                    op0=ALU.mult, op1=ALU.subtract,
                )
                eb = bsmall.tile([128, WIN], BF16, tag="eb")
                pcol = stats.tile([128, 1], F32, tag="pcol")
                nc.scalar.activation(
                    eb, ub, AF.Exp, scale=30.0 / SIMMUL, bias=bias_m150, accum_out=pcol
                )
                u2 = bsmall.tile([128, WIN], F32, tag="u2")
                nc.vector.scalar_tensor_tensor(
                    u2, in0=mt_t, scalar=-1280.0, in1=bp, op0=ALU.mult, op1=ALU.add
                )
                e2 = bsmall.tile([128, WIN], BF16, tag="e2")
                wcol = stats.tile([128, 1], F32, tag="wcol")
                nc.scalar.activation(
                    e2, u2, AF.Exp, scale=30.0 / SIMMUL, accum_out=wcol
                )

                nsum = nsp.tile([128, 12], F32, tag="nsum")
                cnt = 0
                kills = _kill_ranges(m)
                for g in range(NG):
                    ps = psmain.tile([128, GW], F32, tag="ps")
                    for ks in range(2):
                        for half in range(2):
                            nc.tensor.matmul(
                                ps[:, bass.ts(half, 512)],
                                ft8[:, 2 * ks : 2 * ks + 2,
                                    PAD + 128 * m : PAD + 128 * m + 128],
                                ft8[:, 2 * ks : 2 * ks + 2,
                                    PAD + GW * g + 512 * half : PAD + GW * g + 512 * (half + 1)],
                                start=(ks == 0),
                                stop=(ks == 1),
                                perf_mode=DR,
                            )
                    e = epool.tile([128, GW], BF16, tag="e")
                    for a, b in _pieces(g, kills):
                        nc.scalar.activation(
                            e[:, a:b], ps[:, a:b], AF.Exp, scale=30.0 / SIMMUL,
                            accum_out=nsum[:, cnt : cnt + 1],
                        )
                        cnt += 1

                nsr = stats.tile([128, 1], F32, tag="nsr")
                nc.vector.reduce_sum(nsr, nsum[:, 0:cnt], axis=mybir.AxisListType.X)
                ncol = stats.tile([128, 1], F32, tag="ncol")
                nc.vector.tensor_tensor(ncol, nsr, wcol, op=ALU.add)
                nc.vector.tensor_tensor(pns[:, m : m + 1], pcol, ncol, op=ALU.mult)

            losses = singles.tile([128, MT], F32, name="losses")
            nc.scalar.activation(losses, pns, AF.Ln, bias=bias_1)
            lsum = singles.tile([128, 1], F32, name="lsum")
            nc.vector.reduce_sum(lsum, losses, axis=mybir.AxisListType.X)
            nc.sync.dma_start(out=o_loss, in_=lsum)

    nc.compile()
    return nc


def _prep_inputs(feat: np.ndarray, label: np.ndarray):
    perm = np.argsort(label, kind="stable")
    lab = np.asarray(label)[perm].astype(np.int64)
    f = np.asarray(feat, dtype=np.float32)[perm]
    n = np.maximum(np.linalg.norm(f, axis=1, keepdims=True), 1e-8)
    ft8_full = ((f / n) * F8SCALE).astype(ml_dtypes.float8_e4m3fn)

    starts = np.searchsorted(lab, lab, side="left")
    ends = np.searchsorted(lab, lab, side="right")
    ts_g = (np.arange(N) // 128) * 128
    assert (starts >= ts_g - PAD).all() and (ends <= ts_g + 256).all(), (
        "label group exceeds band window"
    )

    widx = ((np.arange(R) // 128 * 128)[:, None] - PAD + np.arange(WIN)[None, :]) % N
    in_maps = []
    for c in range(NCORES):
        sh = c * R
        ftr = np.concatenate([ft8_full[sh:], ft8_full[:sh]], axis=0)
        labr = np.concatenate([lab[sh:], lab[:sh]])
        t4 = ftr.T.reshape(4, 128, N).transpose(1, 0, 2)  # [128,4,N]
        ftp = np.concatenate(
            [t4[:, :, N - PAD :], t4, t4[:, :, :PAD]], axis=2
        ).reshape(128, 4 * NP_)
        mask = (labr[:R, None] == labr[widx]).astype(ml_dtypes.float8_e4m3fn)
        # mask rows tiled to partition-major: [128, MT*WIN]
        maskp = mask.reshape(MT, 128, WIN).transpose(1, 0, 2).reshape(128, MT * WIN)
        in_maps.append({"x_in": np.ascontiguousarray(np.concatenate([ftp, maskp], axis=1))})
    return in_maps


def kernel(feat: np.ndarray, label: np.ndarray) -> np.ndarray:
    feat = np.asarray(feat, dtype=np.float32)
    label = np.asarray(label)
    assert feat.shape == (N, D) and label.shape == (N,)

    in_maps = _prep_inputs(feat, label)

    if "nc" not in _CACHED:
        _CACHED["nc"] = _build_nc()
    nc = _CACHED["nc"]

    res = run_bass_kernel_spmd(nc, in_maps, core_ids=list(range(NCORES)))
    total = sum(float(res.results[c]["o_loss"].sum()) for c in range(NCORES))
    return np.float32(total / N)


# revision 17
# speedup vs baseline: 2.8436x; 1.8131x over previous
"""v4: host-side normalize/fp8/transpose/rotation; device = static O(N^2) loop.

Per-core inputs are pre-rotated so core c's rows are always local tiles
0..MT-1 and every positive pair falls in a static 384-wide band per tile.
Dense negatives: fp8 DoubleRow matmul -> exp directly from PSUM (window
columns excluded by statically splitting the activation). Band: positive
logsumexp (with margin) + window-negative sum via fp8 mask. Loss combine:
ln(1 + pcol*ncol) == softplus(logit_p + logit_n).
"""
import os
import numpy as np
import ml_dtypes

import concourse.bass as bass
import concourse.bacc as bacc
import concourse.mybir as mybir
import concourse.tile as tile
from concourse.bass_utils import run_bass_kernel_spmd

F32 = mybir.dt.float32
BF16 = mybir.dt.bfloat16
F8 = mybir.dt.float8e4
AF = mybir.ActivationFunctionType
ALU = mybir.AluOpType
DR = mybir.MatmulPerfMode.DoubleRow

F8SCALE = 16.0
SIMMUL = F8SCALE * F8SCALE  # 256: bp/ps hold 256*sim

N, D = 8192, 512
NCORES = int(os.environ.get("BASS_NCORES", "1"))
R = N // NCORES
MT = R // 128
PAD = 128
WIN = 384
NP_ = N + 2 * PAD
GW = 1024
NG = N // GW
REPEAT = int(os.environ.get("BASS_REPEAT", "1"))

_CACHED = {}


def _kill_ranges(m):
    """Window [128m-PAD, 128m+256) as cyclic range(s) within [0, N)."""
    s = (128 * m - PAD) % N
    e = s + WIN
    if e <= N:
        return [(s, e)]
    return [(s, N), (0, e - N)]


def _pieces(g, kills):
    lo, hi = g * GW, (g + 1) * GW
    pts = [lo, hi]
    for a, b in kills:
        if a < hi and b > lo:
            pts += [max(a, lo), min(b, hi)]
    pts = sorted(set(pts))
    out = []
    for a, b in zip(pts[:-1], pts[1:]):
        if not any(ka <= a and b <= kb for ka, kb in kills):
            out.append((a - lo, b - lo))
    return out


def _build_nc():
    nc = bacc.Bacc(
        "TRN2", target_bir_lowering=False, debug=False, num_devices=NCORES,
        enable_partition_id=False,
    )

    FTW = 4 * NP_  # 33792: transposed fp8 features, then MT*WIN mask columns
    x_in = nc.dram_tensor("x_in", [128, FTW + MT * WIN], F8, kind="ExternalInput").ap()
    ftv = x_in[:, 0:FTW].rearrange("p (k c) -> p k c", k=4)
    o_loss = nc.dram_tensor("o_loss", [128, 1], F32, kind="ExternalOutput").ap()

    with tile.TileContext(nc) as tc:
        with (
            tc.tile_pool(name="singles", bufs=1) as singles,
            tc.tile_pool(name="maskp", bufs=2) as maskp,
            tc.tile_pool(name="ep", bufs=3) as epool,
            tc.tile_pool(name="bsmall", bufs=2) as bsmall,
            tc.tile_pool(name="stats", bufs=8) as stats,
            tc.tile_pool(name="nsp", bufs=2) as nsp,
            tc.tile_pool(name="psmain", bufs=3, space="PSUM") as psmain,
            tc.tile_pool(name="psband", bufs=2, space="PSUM") as psband,
        ):
            bias_m150 = singles.tile([128, 1], F32, name="bias_m150")
            nc.vector.memset(bias_m150, -150.0)
            bias_1 = singles.tile([128, 1], F32, name="bias_1")
            nc.vector.memset(bias_1, 1.0)

            ft8 = singles.tile([128, 4, NP_], F8, name="ft8")
            NCH = 8
            CH = NP_ // NCH  # 1040
            for j in range(NCH):
                nc.sync.dma_start(
                    out=ft8[:, :, j * CH : (j + 1) * CH],
                    in_=ftv[:, :, j * CH : (j + 1) * CH],
                )
            pns = singles.tile([128, MT], F32, name="pns")

            for m in [mm for _ in range(REPEAT) for mm in range(MT)]:
                mt_t = maskp.tile([128, WIN], F8, tag="mask")
                nc.sync.dma_start(
                    out=mt_t, in_=x_in[:, FTW + m * WIN : FTW + (m + 1) * WIN]
                )

                bp = psband.tile([128, WIN], F32, tag="bp")
                for ks in range(2):
                    nc.tensor.matmul(
                        bp,
                        ft8[:, 2 * ks : 2 * ks + 2, PAD + 128 * m : PAD + 128 * m + 128],
                        ft8[:, 2 * ks : 2 * ks + 2, 128 * m : 128 * m + WIN],
                        start=(ks == 0),
                        stop=(ks == 1),
                        perf_mode=DR,
                    )
                ub = bsmall.tile([128, WIN], F32, tag="ub")
                nc.vector.scalar_tensor_tensor(
                    ub, in0=mt_t, scalar=5.3 * SIMMUL, in1=bp,
                    op0=ALU.mult, op1=ALU.subtract,
                )
                eb = bsmall.tile([128, WIN], BF16, tag="eb")
                pcol = stats.tile([128, 1], F32, tag="pcol")
                nc.scalar.activation(
                    eb, ub, AF.Exp, scale=30.0 / SIMMUL, bias=bias_m150, accum_out=pcol
                )
                u2 = bsmall.tile([128, WIN], F32, tag="u2")
                nc.vector.scalar_tensor_tensor(
                    u2, in0=mt_t, scalar=-1280.0, in1=bp, op0=ALU.mult, op1=ALU.add
                )
                e2 = bsmall.tile([128, WIN], BF16, tag="e2")
                wcol = stats.tile([128, 1], F32, tag="wcol")
                nc.scalar.activation(
                    e2, u2, AF.Exp, scale=30.0 / SIMMUL, accum_out=wcol
                )

                nsum = nsp.tile([128, 12], F32, tag="nsum")
                cnt = 0
                kills = _kill_ranges(m)
                for g in range(NG):
                    ps = psmain.tile([128, GW], F32, tag="ps")
                    for ks in range(2):
                        for half in range(2):
                            nc.tensor.matmul(
                                ps[:, bass.ts(half, 512)],
                                ft8[:, 2 * ks : 2 * ks + 2,
                                    PAD + 128 * m : PAD + 128 * m + 128],
                                ft8[:, 2 * ks : 2 * ks + 2,
                                    PAD + GW * g + 512 * half : PAD + GW * g + 512 * (half + 1)],
                                start=(ks == 0),
                                stop=(ks == 1),
                                perf_mode=DR,
                            )
                    e = epool.tile([128, GW], BF16, tag="e")
                    for a, b in _pieces(g, kills):
                        nc.scalar.activation(
                            e[:, a:b], ps[:, a:b], AF.Exp, scale=30.0 / SIMMUL,
                            accum_out=nsum[:, cnt : cnt + 1],
                        )
                        cnt += 1

                nsr = stats.tile([128, 1], F32, tag="nsr")
                nc.vector.reduce_sum(nsr, nsum[:, 0:cnt], axis=mybir.AxisListType.X)
                ncol = stats.tile([128, 1], F32, tag="ncol")
                nc.vector.tensor_tensor(ncol, nsr, wcol, op=ALU.add)
                nc.vector.tensor_tensor(pns[:, m : m + 1], pcol, ncol, op=ALU.mult)

            losses = singles.tile([128, MT], F32, name="losses")
            nc.scalar.activation(losses, pns, AF.Ln, bias=bias_1)
            lsum = singles.tile([128, 1], F32, name="lsum")
            nc.vector.reduce_sum(lsum, losses, axis=mybir.AxisListType.X)
            nc.sync.dma_start(out=o_loss, in_=lsum)

    nc.compile()
    return nc


def _prep_inputs(feat: np.ndarray, label: np.ndarray):
    perm = np.argsort(label, kind="stable")
    lab = np.asarray(label)[perm].astype(np.int64)
    f = np.asarray(feat, dtype=np.float32)[perm]
    n = np.maximum(np.linalg.norm(f, axis=1, keepdims=True), 1e-8)
    ft8_full = ((f / n) * F8SCALE).astype(ml_dtypes.float8_e4m3fn)

    starts = np.searchsorted(lab, lab, side="left")
    ends = np.searchsorted(lab, lab, side="right")
    ts_g = (np.arange(N) // 128) * 128
    assert (starts >= ts_g - PAD).all() and (ends <= ts_g + 256).all(), (
        "label group exceeds band window"
    )

    widx = ((np.arange(R) // 128 * 128)[:, None] - PAD + np.arange(WIN)[None, :]) % N
    in_maps = []
    for c in range(NCORES):
        sh = c * R
        ftr = np.concatenate([ft8_full[sh:], ft8_full[:sh]], axis=0)
        labr = np.concatenate([lab[sh:], lab[:sh]])
        t4 = ftr.T.reshape(4, 128, N).transpose(1, 0, 2)  # [128,4,N]
        ftp = np.concatenate(
            [t4[:, :, N - PAD :], t4, t4[:, :, :PAD]], axis=2
        ).reshape(128, 4 * NP_)
        mask = (labr[:R, None] == labr[widx]).astype(ml_dtypes.float8_e4m3fn)
        # mask rows tiled to partition-major: [128, MT*WIN]
        maskp = mask.reshape(MT, 128, WIN).transpose(1, 0, 2).reshape(128, MT * WIN)
        in_maps.append({"x_in": np.ascontiguousarray(np.concatenate([ftp, maskp], axis=1))})
    return in_maps


def kernel(feat: np.ndarray, label: np.ndarray) -> np.ndarray:
    feat = np.asarray(feat, dtype=np.float32)
    label = np.asarray(label)
    assert feat.shape == (N, D) and label.shape == (N,)

    in_maps = _prep_inputs(feat, label)

    if "nc" not in _CACHED:
        _CACHED["nc"] = _build_nc()
    nc = _CACHED["nc"]

    res = run_bass_kernel_spmd(nc, in_maps, core_ids=list(range(NCORES)))
    total = sum(float(res.results[c]["o_loss"].sum()) for c in range(NCORES))
    return np.float32(total / N)


# revision 24
# speedup vs baseline: 2.9634x; 1.0421x over previous
"""v4: host-side normalize/fp8/transpose/rotation; device = static O(N^2) loop.

Per-core inputs are pre-rotated so core c's rows are always local tiles
0..MT-1 and every positive pair falls in a static 384-wide band per tile.
Dense negatives: fp8 DoubleRow matmul -> exp directly from PSUM (window
columns excluded by statically splitting the activation). Band: positive
logsumexp (with margin) + window-negative sum via fp8 mask. Loss combine:
ln(1 + pcol*ncol) == softplus(logit_p + logit_n).
"""
import os
import numpy as np
import ml_dtypes

import concourse.bass as bass
import concourse.bacc as bacc
import concourse.mybir as mybir
import concourse.tile as tile
from concourse.bass_utils import run_bass_kernel_spmd

F32 = mybir.dt.float32
BF16 = mybir.dt.bfloat16
F8 = mybir.dt.float8e4
AF = mybir.ActivationFunctionType
ALU = mybir.AluOpType
DR = mybir.MatmulPerfMode.DoubleRow

F8SCALE = 16.0
SIMMUL = F8SCALE * F8SCALE  # 256: bp/ps hold 256*sim

N, D = 8192, 512
NCORES = int(os.environ.get("BASS_NCORES", "1"))
R = N // NCORES
MT = R // 128
PAD = 128
WIN = 384
NP_ = N + 2 * PAD
GW = 1024
NG = N // GW
REPEAT = int(os.environ.get("BASS_REPEAT", "1"))
SWIL = os.environ.get("BASS_SWIL", "0") == "1"
DRSW = mybir.MatmulPerfMode.DoubleRowSwInterleave

_CACHED = {}


def _kill_ranges(m):
    """Window [128m-PAD, 128m+256) as cyclic range(s) within [0, N)."""
    s = (128 * m - PAD) % N
    e = s + WIN
    if e <= N:
        return [(s, e)]
    return [(s, N), (0, e - N)]


def _pieces(g, kills):
    lo, hi = g * GW, (g + 1) * GW
    pts = [lo, hi]
    for a, b in kills:
        if a < hi and b > lo:
            pts += [max(a, lo), min(b, hi)]
    pts = sorted(set(pts))
    out = []
    for a, b in zip(pts[:-1], pts[1:]):
        if not any(ka <= a and b <= kb for ka, kb in kills):
            out.append((a - lo, b - lo))
    return out


def _build_nc():
    nc = bacc.Bacc(
        "TRN2", target_bir_lowering=False, debug=False, num_devices=NCORES,
        enable_partition_id=False,
    )

    FTW = 4 * NP_  # 33792: transposed fp8 features, then MT*WIN mask columns
    WW = MT * 2 * 256 if SWIL else 0  # interleaved DR weights per (tile, ks)
    x_in = nc.dram_tensor(
        "x_in", [128, FTW + MT * WIN + WW], F8, kind="ExternalInput"
    ).ap()
    ftv = x_in[:, 0:FTW].rearrange("p (k c) -> p k c", k=4)
    o_loss = nc.dram_tensor("o_loss", [128, 1], F32, kind="ExternalOutput").ap()

    with tile.TileContext(nc) as tc:
        with (
            tc.tile_pool(name="singles", bufs=1) as singles,
            tc.tile_pool(name="maskp", bufs=2) as maskp,
            tc.tile_pool(name="ep", bufs=3) as epool,
            tc.tile_pool(name="bsmall", bufs=2) as bsmall,
            tc.tile_pool(name="stats", bufs=8) as stats,
            tc.tile_pool(name="nsp", bufs=2) as nsp,
            tc.tile_pool(name="psmain", bufs=3, space="PSUM") as psmain,
            tc.tile_pool(name="psband", bufs=2, space="PSUM") as psband,
        ):
            bias_m150 = singles.tile([128, 1], F32, name="bias_m150")
            nc.vector.memset(bias_m150, -150.0)
            bias_1 = singles.tile([128, 1], F32, name="bias_1")
            nc.vector.memset(bias_1, 1.0)

            ft8 = singles.tile([128, 4, NP_], F8, name="ft8")
            NCH = 8
            CH = NP_ // NCH  # 1040
            for j in range(NCH):
                nc.sync.dma_start(
                    out=ft8[:, :, j * CH : (j + 1) * CH],
                    in_=ftv[:, :, j * CH : (j + 1) * CH],
                )
            pns = singles.tile([128, MT], F32, name="pns")

            if SWIL:
                w8 = singles.tile([128, MT * 2, 128, 2], F8, name="w8")
                wv = x_in[:, FTW + MT * WIN :].rearrange(
                    "p (i b a) -> p i b a", i=MT * 2, b=128
                )
                WCH = MT // 2
                for j in range(4):
                    nc.sync.dma_start(
                        out=w8[:, j * WCH : (j + 1) * WCH, :, :],
                        in_=wv[:, j * WCH : (j + 1) * WCH, :, :],
                    )

            def lhsT(m, ks):
                if SWIL:
                    return w8[:, 2 * m + ks, :, :]
                return ft8[:, 2 * ks : 2 * ks + 2, PAD + 128 * m : PAD + 128 * m + 128]

            PM = DRSW if SWIL else DR

            for m in [mm for _ in range(REPEAT) for mm in range(MT)]:
                mt_t = maskp.tile([128, WIN], F8, tag="mask")
                nc.sync.dma_start(
                    out=mt_t, in_=x_in[:, FTW + m * WIN : FTW + (m + 1) * WIN]
                )

                bp = psband.tile([128, WIN], F32, tag="bp")
                for ks in range(2):
                    nc.tensor.matmul(
                        bp,
                        lhsT(m, ks),
                        ft8[:, 2 * ks : 2 * ks + 2, 128 * m : 128 * m + WIN],
                        start=(ks == 0),
                        stop=(ks == 1),
                        perf_mode=PM,
                    )
                ub = bsmall.tile([128, WIN], F32, tag="ub")
                nc.vector.scalar_tensor_tensor(
                    ub, in0=mt_t, scalar=5.3 * SIMMUL, in1=bp,
                    op0=ALU.mult, op1=ALU.subtract,
                )
                eb = bsmall.tile([128, WIN], BF16, tag="eb")
                pcol = stats.tile([128, 1], F32, tag="pcol")
                nc.scalar.activation(
                    eb, ub, AF.Exp, scale=30.0 / SIMMUL, bias=bias_m150, accum_out=pcol
                )
                u2 = bsmall.tile([128, WIN], F32, tag="u2")
                nc.vector.scalar_tensor_tensor(
                    u2, in0=mt_t, scalar=-1280.0, in1=bp, op0=ALU.mult, op1=ALU.add
                )
                e2 = bsmall.tile([128, WIN], BF16, tag="e2")
                wcol = stats.tile([128, 1], F32, tag="wcol")
                nc.scalar.activation(
                    e2, u2, AF.Exp, scale=30.0 / SIMMUL, accum_out=wcol
                )

                nsum = nsp.tile([128, 12], F32, tag="nsum")
                cnt = 0
                kills = _kill_ranges(m)
                for g in range(NG):
                    ps = psmain.tile([128, GW], F32, tag="ps")
                    for ks in range(2):
                        for half in range(2):
                            nc.tensor.matmul(
                                ps[:, bass.ts(half, 512)],
                                lhsT(m, ks),
                                ft8[:, 2 * ks : 2 * ks + 2,
                                    PAD + GW * g + 512 * half : PAD + GW * g + 512 * (half + 1)],
                                start=(ks == 0),
                                stop=(ks == 1),
                                perf_mode=PM,
                            )
                    e = epool.tile([128, GW], BF16, tag="e")
                    for a, b in _pieces(g, kills):
                        nc.scalar.activation(
                            e[:, a:b], ps[:, a:b], AF.Exp, scale=30.0 / SIMMUL,
                            accum_out=nsum[:, cnt : cnt + 1],
                        )
                        cnt += 1

                nsr = stats.tile([128, 1], F32, tag="nsr")
                nc.vector.reduce_sum(nsr, nsum[:, 0:cnt], axis=mybir.AxisListType.X)
                ncol = stats.tile([128, 1], F32, tag="ncol")
                nc.vector.tensor_tensor(ncol, nsr, wcol, op=ALU.add)
                nc.vector.tensor_tensor(pns[:, m : m + 1], pcol, ncol, op=ALU.mult)

            losses = singles.tile([128, MT], F32, name="losses")
            nc.scalar.activation(losses, pns, AF.Ln, bias=bias_1)
            lsum = singles.tile([128, 1], F32, name="lsum")
            nc.vector.reduce_sum(lsum, losses, axis=mybir.AxisListType.X)
            nc.sync.dma_start(out=o_loss, in_=lsum)

    nc.compile()
    return nc


def _prep_inputs(feat: np.ndarray, label: np.ndarray):
    perm = np.argsort(label, kind="stable")
    lab = np.asarray(label)[perm].astype(np.int64)
    f = np.asarray(feat, dtype=np.float32)[perm]
    n = np.maximum(np.linalg.norm(f, axis=1, keepdims=True), 1e-8)
    ft8_full = ((f / n) * F8SCALE).astype(ml_dtypes.float8_e4m3fn)

    starts = np.searchsorted(lab, lab, side="left")
    ends = np.searchsorted(lab, lab, side="right")
    ts_g = (np.arange(N) // 128) * 128
    assert (starts >= ts_g - PAD).all() and (ends <= ts_g + 256).all(), (
        "label group exceeds band window"
    )

    widx = ((np.arange(R) // 128 * 128)[:, None] - PAD + np.arange(WIN)[None, :]) % N
    in_maps = []
    for c in range(NCORES):
        sh = c * R
        ftr = np.concatenate([ft8_full[sh:], ft8_full[:sh]], axis=0)
        labr = np.concatenate([lab[sh:], lab[:sh]])
        t4 = ftr.T.reshape(4, 128, N).transpose(1, 0, 2)  # [128,4,N]
        ftp = np.concatenate(
            [t4[:, :, N - PAD :], t4, t4[:, :, :PAD]], axis=2
        ).reshape(128, 4 * NP_)
        mask = (labr[:R, None] == labr[widx]).astype(ml_dtypes.float8_e4m3fn)
        # mask rows tiled to partition-major: [128, MT*WIN]
        maskp = mask.reshape(MT, 128, WIN).transpose(1, 0, 2).reshape(128, MT * WIN)
        parts = [ftp, maskp]
        if SWIL:
            # DoubleRowSwInterleave weights: per (m, ks) a [128, 256] block
            # laid out [A127, B127, ..., A0, B0] (pairs interleaved, columns
            # reversed) where A/B are the k-subtile pair's weight columns.
            w = np.empty((128, MT * 2, 2, 128), dtype=ml_dtypes.float8_e4m3fn)
            for m in range(MT):
                for ks in range(2):
                    a = t4[:, 2 * ks, 128 * m : 128 * (m + 1)]
                    b = t4[:, 2 * ks + 1, 128 * m : 128 * (m + 1)]
                    w[:, 2 * m + ks, 0, :] = a[:, ::-1]
                    w[:, 2 * m + ks, 1, :] = b[:, ::-1]
            # stored interleaved: position 2t = A[127-t], 2t+1 = B[127-t]
            wil = w.transpose(0, 1, 3, 2).reshape(128, MT * 2 * 256)
            parts.append(wil)
        in_maps.append({"x_in": np.ascontiguousarray(np.concatenate(parts, axis=1))})
    return in_maps


def kernel(feat: np.ndarray, label: np.ndarray) -> np.ndarray:
    feat = np.asarray(feat, dtype=np.float32)
    label = np.asarray(label)
    assert feat.shape == (N, D) and label.shape == (N,)

    in_maps = _prep_inputs(feat, label)

    if "nc" not in _CACHED:
        _CACHED["nc"] = _build_nc()
    nc = _CACHED["nc"]

    res = run_bass_kernel_spmd(nc, in_maps, core_ids=list(range(NCORES)))
    total = sum(float(res.results[c]["o_loss"].sum()) for c in range(NCORES))
    return np.float32(total / N)
